# revision 1
# baseline (speedup 1.0000x reference)
"""Distortion-loss (eff_distloss) Bass kernel for Trainium2, 8 NeuronCores.

Inputs (full): weights/distances/intervals, each [262144, 128] f32.
Output: scalar f32 loss.

Math: per ray (w, m, s in R^128):
  uni = sum_j s_j w_j^2
  bi  = sum_{j>k} w_j w_k (m_j - m_k) = wm^T (SL - SU) w,  wm = w*m,
        SL/SU strictly lower/upper triangular ones.
  loss = 0.01 * mean_rays(uni/3 + 2*bi)

Total bi over a batch of rays = <A^T, W^T WM>_F with A = SL - SU (constant)
and W^T WM a Gram matrix accumulated over rays; uni = sum diag(W^T SW),
sw = s*w. The O(N) cumsum formulation is never materialized. On the PE,
each 128-ray block is ONE ldweights (stationary w) + ONE 256-wide matmul
streaming [wm | sw] into a single [128, 256] PSUM accumulator holding both
Gram matrices side by side.

Sharding: pure data-parallel over the ray axis, B=262144 -> 32768 rays on
each of the 8 cores. Each core returns 128+128 per-partition partial sums
(bi and uni); the host does the final tiny reduction and scaling.

Raw-bass implementation (no Tile): three engine programs (sync=DMA,
vector=elementwise bf16 products, tensor=Gram matmuls) with NB-deep ring
buffers. DMA completion uses one semaphore per ring slot with full-tile
thresholds (a single counting semaphore over interleaved multi-engine
DMAs can hit a threshold through shards of later transfers, so partial
thresholds are unsound). The schedule ends with two half tiles and a
stream-split, quarter-granular final tile so the PE/DVE tail pipelines
behind the last DMAs instead of serializing after them.
"""

import numpy as np

import concourse.bass as bass
import concourse.mybir as mybir
from concourse.bass_utils import run_bass_kernel_spmd

B, N = 262144, 128
NCORES = 8
B_PER = B // NCORES  # 32768 rays per core
P = 128  # SBUF partitions = rays per matmul block
RMAX = 16  # rays per partition in a full tile
# 15 full tiles + 2 half tiles = 15*16 + 2*8 = 256 ray-blocks per core
SCHED = [16] * 15 + [8, 8]
assert sum(SCHED) * P == B_PER
T = len(SCHED)
FREE = RMAX * N  # ring slot size (f32 elements per partition)
NB = 4  # ring depth
NQ = 4  # last-tile compute split

F32 = mybir.dt.float32
BF16 = mybir.dt.bfloat16

LOSS_WEIGHT = 0.01

_cached = {}


def _build_nc() -> bass.Bass:
    nc = bass.Bass(trn_type="TRN2", monotonic_sem_count=0)

    w_h = nc.declare_dram_parameter("weights", [B_PER, N], F32, isOutput=False)
    m_h = nc.declare_dram_parameter("distances", [B_PER, N], F32, isOutput=False)
    s_h = nc.declare_dram_parameter("intervals", [B_PER, N], F32, isOutput=False)
    ai_h = nc.declare_dram_parameter("aimat", [P, 2 * N], F32, isOutput=False)
    out_h = nc.declare_dram_parameter("partials", [P, 2], F32, isOutput=True)

    # per-tile DRAM views: tile i covers rays [off, off + P*R_i)
    offs = [0]
    for r in SCHED:
        offs.append(offs[-1] + P * r)

    def dram_view(h, i):
        r = SCHED[i]
        return h[offs[i] : offs[i + 1], :].rearrange("(p r) n -> p (r n)", p=P, r=r)

    # dve_sem increments: 3 per tile for tiles 0..T-2, then 3*NQ for the
    # split last tile, then 1 for the finale.
    def dve_after_tile(i):
        return 3 * (i + 1) if i < T - 1 else 3 * (T - 1) + 3 * NQ

    DVE_FINAL = dve_after_tile(T - 1) + 1

    R_LAST = SCHED[-1]
    QF = R_LAST * N // NQ  # f32 elements per quarter of the last tile
    QR = R_LAST // NQ  # ray-blocks per quarter

    import contextlib

    with contextlib.ExitStack() as ctx:
        ec = ctx.enter_context
        w_sb = ec(nc.sbuf_tensor([P, NB * FREE], F32))
        m_sb = ec(nc.sbuf_tensor([P, NB * FREE], F32))
        s_sb = ec(nc.sbuf_tensor([P, NB * FREE], F32))
        # [wm | sw] interleaved per ray block: block r occupies columns
        # [r*2N, r*2N + 2N) of the slot, wm in the low half, sw in the high
        ws_sb = ec(nc.sbuf_tensor([P, NB * 2 * FREE], BF16))
        wb_sb = ec(nc.sbuf_tensor([P, NB * FREE], BF16))
        ai_sb = ec(nc.sbuf_tensor([P, 2 * N], F32))
        out_sb = ec(nc.sbuf_tensor([P, 2], F32))
        tr_sb = ec(nc.sbuf_tensor([P, 2 * N], F32))
        g12_ps = ec(nc.psum_tensor([P, 2 * N], F32))  # [W^T WM | W^T SW]
        slot_sem = [ec(nc.semaphore(f"dma_slot{i}")) for i in range(NB)]
        lw_sem = ec(nc.semaphore("dma_lw"))
        lm_sem = ec(nc.semaphore("dma_lm"))
        ls_sem = [ec(nc.semaphore(f"dma_ls{q}")) for q in range(NQ)]
        dve_sem = ec(nc.semaphore("dve_sem"))
        pe_sem = ec(nc.semaphore("pe_sem"))
        block = ec(nc.Block(no_gpsimd_drain=True))

        def sl(i, n_el=None):
            base = (i % NB) * FREE
            return slice(base, base + (SCHED[i] * N if n_el is None else n_el))

        def f32_3d(t_sb, i, q=None):
            # [P, R, N] view of an io slot (or one quarter of the last slot)
            if q is None:
                return t_sb[:, sl(i)].rearrange("p (r n) -> p r n", n=N)
            base = (i % NB) * FREE
            return t_sb[:, base + q * QF : base + (q + 1) * QF].rearrange(
                "p (r n) -> p r n", n=N
            )

        def ws_3d(i, half, q=None):
            # [P, R, N] strided view into the [wm | sw] pair layout
            base2 = (i % NB) * 2 * FREE
            if q is None:
                r = SCHED[i]
                v = ws_sb[:, base2 : base2 + 2 * r * N]
            else:
                v = ws_sb[:, base2 + q * 2 * QF : base2 + (q + 1) * 2 * QF]
            v = v.rearrange("p (r x) -> p r x", x=2 * N)
            return v[:, :, half * N : (half + 1) * N]

        @block.sync
        def _(sync: bass.BassEngine):
            for i in range(T):
                k = i % NB
                if i >= NB:
                    # io ring slot (i-NB) fully consumed by DVE
                    sync.wait_ge(dve_sem, dve_after_tile(i - NB))
                if i == T - 1:
                    # consts ride the last tile's stream sems; the s stream
                    # is quartered so the DVE tail can chase it
                    sync.dma_start(out=ai_sb[:], in_=ai_h[:, :]).then_inc(lw_sem, 16)
                    sync.dma_start(out=w_sb[:, sl(i)], in_=dram_view(w_h, i)).then_inc(
                        lw_sem, 16
                    )
                    sync.dma_start(out=m_sb[:, sl(i)], in_=dram_view(m_h, i)).then_inc(
                        lm_sem, 16
                    )
                    base = (i % NB) * FREE
                    s_last = dram_view(s_h, i)
                    for q in range(NQ):
                        sync.dma_start(
                            out=s_sb[:, base + q * QF : base + (q + 1) * QF],
                            in_=s_last[:, q * QF : (q + 1) * QF],
                        ).then_inc(ls_sem[q], 16)
                else:
                    sync.dma_start(out=w_sb[:, sl(i)], in_=dram_view(w_h, i)).then_inc(
                        slot_sem[k], 16
                    )
                    sync.dma_start(out=m_sb[:, sl(i)], in_=dram_view(m_h, i)).then_inc(
                        slot_sem[k], 16
                    )
                    sync.dma_start(out=s_sb[:, sl(i)], in_=dram_view(s_h, i)).then_inc(
                        slot_sem[k], 16
                    )
            sync.wait_ge(dve_sem, DVE_FINAL)
            sync.dma_start(out=out_h[:, :], in_=out_sb[:]).then_inc(pe_sem, 16)
            # the out-DMA must fully land before the NEFF ends: an in-flight
            # DMA across the NEFF boundary corrupts runtime state.
            sync.wait_ge(pe_sem, T + 16)

        @block.vector
        def _(vector: bass.BassEngine):
            for i in range(T - 1):
                k = i % NB
                vector.wait_ge(slot_sem[k], 48 * (i // NB + 1))
                if i >= NB:
                    # bf16 ring slot (i-NB) fully consumed by PE
                    vector.wait_ge(pe_sem, i - NB + 1)
                vector.tensor_copy(out=wb_sb[:, sl(i)], in_=w_sb[:, sl(i)]).then_inc(
                    dve_sem, 1
                )
                vector.tensor_mul(
                    ws_3d(i, 0), f32_3d(w_sb, i), f32_3d(m_sb, i)
                ).then_inc(dve_sem, 1)
                vector.tensor_mul(
                    ws_3d(i, 1), f32_3d(s_sb, i), f32_3d(w_sb, i)
                ).then_inc(dve_sem, 1)
            # last tile, quarter-granular so PE can chase
            i = T - 1
            base = (i % NB) * FREE

            def q_sl(q):
                return slice(base + q * QF, base + (q + 1) * QF)

            vector.wait_ge(lw_sem, 32)  # [A|I] + w(last)
            vector.wait_ge(pe_sem, i - NB + 1)
            for q in range(NQ):
                vector.tensor_copy(
                    out=wb_sb[:, q_sl(q)], in_=w_sb[:, q_sl(q)]
                ).then_inc(dve_sem, 1)
            vector.wait_ge(lm_sem, 16)  # m(last)
            for q in range(NQ):
                vector.tensor_mul(
                    ws_3d(i, 0, q), f32_3d(w_sb, i, q), f32_3d(m_sb, i, q)
                ).then_inc(dve_sem, 1)
            for q in range(NQ):
                vector.wait_ge(ls_sem[q], 16)  # s(last) quarter q
                vector.tensor_mul(
                    ws_3d(i, 1, q), f32_3d(s_sb, i, q), f32_3d(w_sb, i, q)
                ).then_inc(dve_sem, 1)
            # finale: one fused weighted reduction of both Gram halves
            vector.wait_ge(pe_sem, T)
            vector.tensor_mul(tr_sb[:], g12_ps[:], ai_sb[:])
            vector.tensor_reduce(
                out_sb[:, 0:2],
                tr_sb[:].rearrange("p (two n) -> p two n", n=N),
                axis=mybir.AxisListType.X,
                op=mybir.AluOpType.add,
            ).then_inc(dve_sem, 1)

        @block.tensor
        def _(tensor: bass.BassEngine):
            for i in range(T - 1):
                base = (i % NB) * FREE
                base2 = (i % NB) * 2 * FREE
                # one matmul per ray block needs cp + wm + sw (3 incs)
                tensor.wait_ge(dve_sem, 3 * i + 3)
                last_mm = None
                for r in range(SCHED[i]):
                    wblk = slice(base + r * N, base + (r + 1) * N)
                    pblk = slice(base2 + r * 2 * N, base2 + (r + 1) * 2 * N)
                    last_mm = nc.tensor.matmul(
                        out=g12_ps[:],
                        lhsT=wb_sb[:, wblk],
                        rhs=ws_sb[:, pblk],
                        start=(i == 0 and r == 0),
                        stop=False,
                    )
                last_mm.then_inc(pe_sem, 1)
            # last tile: chase the DVE quarters
            i = T - 1
            base = (i % NB) * FREE
            base2 = (i % NB) * 2 * FREE
            b3 = 3 * i
            last_mm = None
            for q in range(NQ):
                # quarter q needs cp_q, wm_q, sw_q (inc b3 + 2*NQ + 1 + q)
                tensor.wait_ge(dve_sem, b3 + 2 * NQ + 1 + q)
                for r in range(QR):
                    rr = q * QR + r
                    wblk = slice(base + rr * N, base + (rr + 1) * N)
                    pblk = slice(base2 + rr * 2 * N, base2 + (rr + 1) * 2 * N)
                    last_mm = nc.tensor.matmul(
                        out=g12_ps[:],
                        lhsT=wb_sb[:, wblk],
                        rhs=ws_sb[:, pblk],
                        start=False,
                        stop=(q == NQ - 1 and r == QR - 1),
                    )
            last_mm.then_inc(pe_sem, 1)

    return nc


def _a2mat() -> np.ndarray:
    # transpose of (SL - SU): the kernel accumulates W^T WM = G1^T, and
    # <A, G1> = <A^T, G1^T>
    a = np.triu(np.ones((N, N), np.float32), 1) - np.tril(
        np.ones((N, N), np.float32), -1
    )
    return np.ascontiguousarray(a, dtype=np.float32)


def _aimat() -> np.ndarray:
    return np.ascontiguousarray(
        np.concatenate([_a2mat(), np.eye(N, dtype=np.float32)], axis=1)
    )


def kernel(weights: np.ndarray, distances: np.ndarray, intervals: np.ndarray):
    if "nc" not in _cached:
        _cached["nc"] = _build_nc()
    nc = _cached["nc"]

    w8 = np.ascontiguousarray(weights, np.float32).reshape(NCORES, B_PER, N)
    m8 = np.ascontiguousarray(distances, np.float32).reshape(NCORES, B_PER, N)
    s8 = np.ascontiguousarray(intervals, np.float32).reshape(NCORES, B_PER, N)
    ai = _aimat()

    in_maps = [
        {
            "weights": w8[i],
            "distances": m8[i],
            "intervals": s8[i],
            "aimat": ai,
        }
        for i in range(NCORES)
    ]
    res = run_bass_kernel_spmd(nc, in_maps, list(range(NCORES))).results

    total_bi = 0.0
    total_uni = 0.0
    for i in range(NCORES):
        p = res[i]["partials"].astype(np.float64)
        total_bi += p[:, 0].sum()
        total_uni += p[:, 1].sum()

    loss = LOSS_WEIGHT * ((total_uni / 3.0) + 2.0 * total_bi) / B
    return np.asarray(loss, dtype=np.float32)



# revision 3
# speedup vs baseline: 1.1788x; 1.1788x over previous
"""Distortion-loss (eff_distloss) Bass kernel for Trainium2, 8 NeuronCores.

Inputs (full): weights/distances/intervals, each [262144, 128] f32.
Output: scalar f32 loss.

Math: per ray (w, m, s in R^128):
  uni = sum_j s_j w_j^2
  bi  = sum_{j>k} w_j w_k (m_j - m_k) = wm^T (SL - SU) w,  wm = w*m,
        SL/SU strictly lower/upper triangular ones.
  loss = 0.01 * mean_rays(uni/3 + 2*bi)

Total bi over a batch of rays = <A^T, W^T WM>_F with A = SL - SU (constant)
and W^T WM a Gram matrix accumulated over rays; uni = sum diag(W^T SW),
sw = s*w. On the PE, each 128-ray block is ONE ldweights (stationary w) +
ONE 256-wide matmul streaming [wm | sw] into a single [128, 256] PSUM
accumulator holding both Gram matrices side by side. The final weights
(2 for bi, 1/3 for uni) are folded into the constant matrix, so the
finale is a single fused multiply-reduce into a [128,1] column, a
ones-vector matmul collapsing it to one scalar, and a 4-byte store.

Sharding: pure data-parallel over the ray axis, B=262144 -> 32768 rays on
each of the 8 cores; the host sums the 8 scalars.

Raw-bass implementation (no Tile): engine programs (sync=HWDGE DMA,
vector=elementwise bf16 products, tensor=Gram matmuls, gpsimd=constant
load via the separate SWDGE ring) with NB-deep ring buffers. DMA
completion uses one semaphore per ring slot with full-tile thresholds.
Tail: w/m of the last two (half) tiles are prefetched earlier in the
FIFO queue so only their s streams land last, split into 2-block chunks
the DVE/PE chase; the output is a single scalar whose DMA carries one
descriptor (a [128,x] store fans into 128 tiny descriptors whose 16
per-engine completion incs straggle ~2us)."""

import numpy as np

import concourse.bass as bass
import concourse.mybir as mybir
from concourse.bass_utils import run_bass_kernel_spmd

B, N = 262144, 128
NCORES = 8
B_PER = B // NCORES  # 32768 rays per core
P = 128  # SBUF partitions = rays per matmul block
RMAX = 16  # rays per partition in a full tile
# 15 full tiles + 2 half tiles = 15*16 + 2*8 = 256 ray-blocks per core
SCHED = [16] * 15 + [8, 8]
assert sum(SCHED) * P == B_PER
T = len(SCHED)
FREE = RMAX * N  # ring slot size (f32 elements per partition)
NB = 4  # ring depth
NQ = 4  # chunks per tail-tile s stream (2 blocks each)

F32 = mybir.dt.float32
BF16 = mybir.dt.bfloat16

LOSS_WEIGHT = 0.01

_cached = {}


def _build_nc() -> bass.Bass:
    nc = bass.Bass(trn_type="TRN2", monotonic_sem_count=0)

    w_h = nc.declare_dram_parameter("weights", [B_PER, N], F32, isOutput=False)
    m_h = nc.declare_dram_parameter("distances", [B_PER, N], F32, isOutput=False)
    s_h = nc.declare_dram_parameter("intervals", [B_PER, N], F32, isOutput=False)
    ai_h = nc.declare_dram_parameter("aimat", [P, 2 * N + 1], F32, isOutput=False)
    out_h = nc.declare_dram_parameter("partials", [1, 1], F32, isOutput=True)

    # per-tile DRAM views: tile i covers rays [off, off + P*R_i)
    offs = [0]
    for r in SCHED:
        offs.append(offs[-1] + P * r)

    def dram_view(h, i):
        r = SCHED[i]
        return h[offs[i] : offs[i + 1], :].rearrange("(p r) n -> p (r n)", p=P, r=r)

    # DVE inc ledger:
    #   tiles 0..T-3: 3 each                                 -> 1..45
    #   cast15, wm15, cast16, wm16                           -> 46..49
    #   sw15 chunks 0..NQ-1, sw16 chunks 0..NQ-1             -> 50..57
    #   finale ttr                                           -> 58
    #   psum-scalar copy                                     -> 59
    def dve_after_tile(i):
        assert i <= T - 3
        return 3 * (i + 1)

    DVE_BASE = 3 * (T - 2)  # 45
    DVE_TTR = DVE_BASE + 4 + 2 * NQ + 1  # 58
    DVE_FINAL = DVE_TTR + 1  # 59

    R_HALF = SCHED[-1]  # 8 blocks in each tail tile
    QR = R_HALF // NQ  # ray-blocks per chunk (2)
    QF = QR * N  # f32 elements per partition per chunk

    # pe_sem ledger: 1 per tile (T total), +1 scalar matmul, +16 out-DMA
    PE_ALL = T  # 17
    PE_SCALAR = T + 1  # 18

    import contextlib

    with contextlib.ExitStack() as ctx:
        ec = ctx.enter_context
        w_sb = ec(nc.sbuf_tensor([P, NB * FREE], F32))
        m_sb = ec(nc.sbuf_tensor([P, NB * FREE], F32))
        s_sb = ec(nc.sbuf_tensor([P, NB * FREE], F32))
        # [wm | sw] interleaved per ray block: block r occupies columns
        # [r*2N, r*2N + 2N) of the slot, wm in the low half, sw in the high
        ws_sb = ec(nc.sbuf_tensor([P, NB * 2 * FREE], BF16))
        wb_sb = ec(nc.sbuf_tensor([P, NB * FREE], BF16))
        ai_sb = ec(nc.sbuf_tensor([P, 2 * N + 1], F32))
        acc_sb = ec(nc.sbuf_tensor([P, 1], F32))
        outs_sb = ec(nc.sbuf_tensor([1, 1], F32))
        tr_sb = ec(nc.sbuf_tensor([P, 2 * N], F32))
        g12_ps = ec(nc.psum_tensor([P, 2 * N], F32))  # [W^T WM | W^T SW]
        sc_ps = ec(nc.psum_tensor([1, 1], F32))
        slot_sem = [ec(nc.semaphore(f"dma_slot{i}")) for i in range(NB)]
        lw_sem = [ec(nc.semaphore(f"dma_lw{t}")) for t in range(2)]
        lm_sem = [ec(nc.semaphore(f"dma_lm{t}")) for t in range(2)]
        ls_sem = [
            [ec(nc.semaphore(f"dma_ls{t}_{q}")) for q in range(NQ)] for t in range(2)
        ]
        ai_sem = ec(nc.semaphore("dma_ai"))
        dve_sem = ec(nc.semaphore("dve_sem"))
        pe_sem = ec(nc.semaphore("pe_sem"))
        block = ec(nc.Block(no_gpsimd_drain=True))

        def sl(i, n_el=None):
            base = (i % NB) * FREE
            return slice(base, base + (SCHED[i] * N if n_el is None else n_el))

        def f32_3d(t_sb, i, q=None):
            # [P, R, N] view of an io slot (or one chunk of a tail slot)
            if q is None:
                return t_sb[:, sl(i)].rearrange("p (r n) -> p r n", n=N)
            base = (i % NB) * FREE
            return t_sb[:, base + q * QF : base + (q + 1) * QF].rearrange(
                "p (r n) -> p r n", n=N
            )

        def ws_3d(i, half, q=None):
            # [P, R, N] strided view into the [wm | sw] pair layout
            base2 = (i % NB) * 2 * FREE
            if q is None:
                r = SCHED[i]
                v = ws_sb[:, base2 : base2 + 2 * r * N]
            else:
                v = ws_sb[:, base2 + q * 2 * QF : base2 + (q + 1) * 2 * QF]
            v = v.rearrange("p (r x) -> p r x", x=2 * N)
            return v[:, :, half * N : (half + 1) * N]

        @block.gpsimd
        def _(g: bass.BassEngine):
            # constants ride the SWDGE ring: zero cost to the HWDGE stream
            g.dma_start(out=ai_sb[:], in_=ai_h[:, :]).then_inc(ai_sem, 16)

        @block.sync
        def _(sync: bass.BassEngine):
            for i in range(T - 2):
                k = i % NB
                if i >= NB:
                    # io ring slot (i-NB) fully consumed by DVE
                    sync.wait_ge(dve_sem, dve_after_tile(i - NB))
                sync.dma_start(out=w_sb[:, sl(i)], in_=dram_view(w_h, i)).then_inc(
                    slot_sem[k], 16
                )
                sync.dma_start(out=m_sb[:, sl(i)], in_=dram_view(m_h, i)).then_inc(
                    slot_sem[k], 16
                )
                sync.dma_start(out=s_sb[:, sl(i)], in_=dram_view(s_h, i)).then_inc(
                    slot_sem[k], 16
                )
            # tail tiles 15/16: w and m ride the queue ahead of the s
            # chunks, so the only data landing at the stream end is s.
            for t, i in enumerate((T - 2, T - 1)):
                sync.wait_ge(dve_sem, dve_after_tile(i - NB))
                sync.dma_start(out=w_sb[:, sl(i)], in_=dram_view(w_h, i)).then_inc(
                    lw_sem[t], 16
                )
                sync.dma_start(out=m_sb[:, sl(i)], in_=dram_view(m_h, i)).then_inc(
                    lm_sem[t], 16
                )
            for t, i in enumerate((T - 2, T - 1)):
                base = (i % NB) * FREE
                s_view = dram_view(s_h, i)
                for q in range(NQ):
                    sync.dma_start(
                        out=s_sb[:, base + q * QF : base + (q + 1) * QF],
                        in_=s_view[:, q * QF : (q + 1) * QF],
                    ).then_inc(ls_sem[t][q], 16)
            sync.wait_ge(dve_sem, DVE_FINAL)
            sync.dma_start(out=out_h[:, :], in_=outs_sb[:]).then_inc(pe_sem, 16)
            # the out-DMA must fully land before the NEFF ends: an in-flight
            # DMA across the NEFF boundary corrupts runtime state.
            sync.wait_ge(pe_sem, PE_SCALAR + 16)

        @block.vector
        def _(vector: bass.BassEngine):
            for i in range(T - 2):
                k = i % NB
                vector.wait_ge(slot_sem[k], 48 * (i // NB + 1))
                if i >= NB:
                    # bf16 ring slot (i-NB) fully consumed by PE
                    vector.wait_ge(pe_sem, i - NB + 1)
                vector.tensor_copy(out=wb_sb[:, sl(i)], in_=w_sb[:, sl(i)]).then_inc(
                    dve_sem, 1
                )
                vector.tensor_mul(
                    ws_3d(i, 0), f32_3d(w_sb, i), f32_3d(m_sb, i)
                ).then_inc(dve_sem, 1)
                vector.tensor_mul(
                    ws_3d(i, 1), f32_3d(s_sb, i), f32_3d(w_sb, i)
                ).then_inc(dve_sem, 1)
            # tail tiles: cast + wm as soon as their (early) loads land
            for t, i in enumerate((T - 2, T - 1)):
                vector.wait_ge(pe_sem, i - NB + 1)
                vector.wait_ge(lw_sem[t], 16)
                vector.tensor_copy(out=wb_sb[:, sl(i)], in_=w_sb[:, sl(i)]).then_inc(
                    dve_sem, 1
                )
                vector.wait_ge(lm_sem[t], 16)
                vector.tensor_mul(
                    ws_3d(i, 0), f32_3d(w_sb, i), f32_3d(m_sb, i)
                ).then_inc(dve_sem, 1)
            # chase the s chunks
            for t, i in enumerate((T - 2, T - 1)):
                for q in range(NQ):
                    vector.wait_ge(ls_sem[t][q], 16)
                    vector.tensor_mul(
                        ws_3d(i, 1, q), f32_3d(s_sb, i, q), f32_3d(w_sb, i, q)
                    ).then_inc(dve_sem, 1)
            # finale: (G * [2A | I/3]) multiply-reduce -> [128,1]
            # (tensor_tensor_reduce would fuse these but fails codegen:
            # "ISA wrong length")
            vector.wait_ge(pe_sem, PE_ALL)
            vector.wait_ge(ai_sem, 16)
            vector.tensor_mul(tr_sb[:], g12_ps[:], ai_sb[:, 0 : 2 * N])
            vector.tensor_reduce(
                acc_sb[:],
                tr_sb[:],
                axis=mybir.AxisListType.X,
                op=mybir.AluOpType.add,
            ).then_inc(dve_sem, 1)
            # collapse to one scalar via the PE, then stage it for the DMA
            vector.wait_ge(pe_sem, PE_SCALAR)
            vector.tensor_copy(out=outs_sb[:], in_=sc_ps[:]).then_inc(dve_sem, 1)

        @block.tensor
        def _(tensor: bass.BassEngine):
            for i in range(T - 2):
                base = (i % NB) * FREE
                base2 = (i % NB) * 2 * FREE
                # one matmul per ray block needs cp + wm + sw (3 incs)
                tensor.wait_ge(dve_sem, 3 * i + 3)
                last_mm = None
                for r in range(SCHED[i]):
                    wblk = slice(base + r * N, base + (r + 1) * N)
                    pblk = slice(base2 + r * 2 * N, base2 + (r + 1) * 2 * N)
                    last_mm = nc.tensor.matmul(
                        out=g12_ps[:],
                        lhsT=wb_sb[:, wblk],
                        rhs=ws_sb[:, pblk],
                        start=(i == 0 and r == 0),
                        stop=False,
                    )
                last_mm.then_inc(pe_sem, 1)
            # tail tiles: chase the DVE chunks
            for t, i in enumerate((T - 2, T - 1)):
                base = (i % NB) * FREE
                base2 = (i % NB) * 2 * FREE
                sw_base = DVE_BASE + 4 + t * NQ  # dve count before this
                last_mm = None
                for r in range(R_HALF):
                    q = r // QR
                    tensor.wait_ge(dve_sem, sw_base + q + 1)
                    wblk = slice(base + r * N, base + (r + 1) * N)
                    pblk = slice(base2 + r * 2 * N, base2 + (r + 1) * 2 * N)
                    last_mm = nc.tensor.matmul(
                        out=g12_ps[:],
                        lhsT=wb_sb[:, wblk],
                        rhs=ws_sb[:, pblk],
                        start=False,
                        stop=(i == T - 1 and r == R_HALF - 1),
                    )
                last_mm.then_inc(pe_sem, 1)
            # ones-weighted column sum: [1,1] scalar in PSUM
            tensor.wait_ge(dve_sem, DVE_TTR)
            nc.tensor.matmul(
                out=sc_ps[:],
                lhsT=acc_sb[:],
                rhs=ai_sb[:, 2 * N : 2 * N + 1],
                start=True,
                stop=True,
            ).then_inc(pe_sem, 1)

    return nc


def _aimat() -> np.ndarray:
    # transpose of (SL - SU): the kernel accumulates W^T WM = G1^T, and
    # <A, G1> = <A^T, G1^T>. The 2x (bi) and 1/3 (uni) loss weights are
    # folded in; the trailing column of ones drives the scalar-collapse
    # matmul.
    a = np.triu(np.ones((N, N), np.float32), 1) - np.tril(
        np.ones((N, N), np.float32), -1
    )
    return np.ascontiguousarray(
        np.concatenate(
            [
                2.0 * a,
                np.eye(N, dtype=np.float32) / 3.0,
                np.ones((N, 1), dtype=np.float32),
            ],
            axis=1,
        )
    )


def kernel(weights: np.ndarray, distances: np.ndarray, intervals: np.ndarray):
    if "nc" not in _cached:
        _cached["nc"] = _build_nc()
    nc = _cached["nc"]

    w8 = np.ascontiguousarray(weights, np.float32).reshape(NCORES, B_PER, N)
    m8 = np.ascontiguousarray(distances, np.float32).reshape(NCORES, B_PER, N)
    s8 = np.ascontiguousarray(intervals, np.float32).reshape(NCORES, B_PER, N)
    ai = _aimat()

    in_maps = [
        {
            "weights": w8[i],
            "distances": m8[i],
            "intervals": s8[i],
            "aimat": ai,
        }
        for i in range(NCORES)
    ]
    res = run_bass_kernel_spmd(nc, in_maps, list(range(NCORES))).results

    total = 0.0
    for i in range(NCORES):
        total += float(res[i]["partials"].astype(np.float64)[0, 0])

    loss = LOSS_WEIGHT * total / B
    return np.asarray(loss, dtype=np.float32)


# revision 4
# speedup vs baseline: 1.1953x; 1.0140x over previous
"""Distortion-loss (eff_distloss) Bass kernel for Trainium2, 8 NeuronCores.

Inputs (full): weights/distances/intervals, each [262144, 128] f32.
Output: scalar f32 loss.

Math: per ray (w, m, s in R^128):
  uni = sum_j s_j w_j^2
  bi  = sum_{j>k} w_j w_k (m_j - m_k) = wm^T (SL - SU) w,  wm = w*m,
        SL/SU strictly lower/upper triangular ones.
  loss = 0.01 * mean_rays(uni/3 + 2*bi)

Total bi over a batch of rays = <A^T, W^T WM>_F with A = SL - SU (constant)
and W^T WM a Gram matrix accumulated over rays; uni = sum diag(W^T SW),
sw = s*w. On the PE, each 128-ray block is ONE ldweights (stationary w) +
ONE 256-wide matmul streaming [wm ; sw] (a [P,2,N] strided rhs over the
slot's contiguous wm/sw halves -- strided DVE writes into an interleaved
layout run at ~96 G elem/s vs ~216 contiguous) into a single [128, 256]
PSUM accumulator holding both Gram matrices. The 2x (bi) and 1/3 (uni)
loss weights are folded into the constant matrix, so the finale is one
multiply+reduce into a [128,1] column, a ones-column matmul collapsing it
to a single scalar, and a 4-byte store (a [128,x] store fans into 128
tiny descriptors whose 16 per-engine completion incs straggle ~2us).

Sharding: pure data-parallel over the ray axis, B=262144 -> 32768 rays on
each of the 8 cores; the host sums the 8 scalars.

Raw-bass implementation (no Tile): engine programs (sync=HWDGE DMA,
vector=elementwise bf16 products, tensor=Gram matmuls, gpsimd=constant
load via the separate SWDGE ring). DMA completion uses one counting
semaphore per (tensor, ring slot) with full-transfer thresholds, so the
DVE starts a tile's cast/wm as soon as w/m land instead of waiting for
the whole tile. Tail: the last three tiles' s streams are chunked (the
w/m of the final two ride the FIFO queue ahead of all s chunks) so the
DVE/PE chase starts ~8us before the stream ends and only a ~1-block
tail remains after the last byte."""

import numpy as np

import concourse.bass as bass
import concourse.mybir as mybir
from concourse.bass_utils import run_bass_kernel_spmd

B, N = 262144, 128
NCORES = 8
B_PER = B // NCORES  # 32768 rays per core
P = 128  # SBUF partitions = rays per matmul block
RMAX = 16  # rays per partition in a full tile
# 15 full tiles + 2 half tiles = 15*16 + 2*8 = 256 ray-blocks per core
SCHED = [16] * 15 + [8, 8]
assert sum(SCHED) * P == B_PER
T = len(SCHED)
FREE = RMAX * N  # ring slot size (f32 elements per partition)
NB = 4  # ring depth
NQ = 4  # s-stream chunks for each of the last three tiles

F32 = mybir.dt.float32
BF16 = mybir.dt.bfloat16

LOSS_WEIGHT = 0.01

_cached = {}


def _build_nc() -> bass.Bass:
    nc = bass.Bass(trn_type="TRN2", monotonic_sem_count=0)

    w_h = nc.declare_dram_parameter("weights", [B_PER, N], F32, isOutput=False)
    m_h = nc.declare_dram_parameter("distances", [B_PER, N], F32, isOutput=False)
    s_h = nc.declare_dram_parameter("intervals", [B_PER, N], F32, isOutput=False)
    ai_h = nc.declare_dram_parameter("aimat", [P, 2 * N + 1], F32, isOutput=False)
    out_h = nc.declare_dram_parameter("partials", [1, 1], F32, isOutput=True)

    # per-tile DRAM views: tile i covers rays [off, off + P*R_i)
    offs = [0]
    for r in SCHED:
        offs.append(offs[-1] + P * r)

    def dram_view(h, i):
        r = SCHED[i]
        return h[offs[i] : offs[i + 1], :].rearrange("(p r) n -> p (r n)", p=P, r=r)

    # DVE inc ledger:
    #   tiles 0..13 : cast+wm+sw, 3 each                  -> 1..42
    #   tile 14     : cast, wm, sw chunks 0..3            -> 43..48
    #   cast15, wm15, cast16, wm16                        -> 49..52
    #   sw15 chunks 0..3, sw16 chunks 0..3                -> 53..60
    #   finale reduce                                     -> 61
    #   psum-scalar copy                                  -> 62
    def dve_after_tile(i):
        assert i <= T - 4
        return 3 * (i + 1)

    DVE_T14 = 3 * (T - 3)  # 42: count before tile 14's ops
    DVE_TAIL = DVE_T14 + 2 + NQ  # 48: count before cast15
    DVE_SW15 = DVE_TAIL + 4  # 52: count before sw15 chunk 0
    DVE_RED = DVE_SW15 + 2 * NQ + 1  # 61: the finale reduce's inc
    DVE_FINAL = DVE_RED + 1  # 62

    R_HALF = SCHED[-1]  # 8 blocks in each tail tile
    # chunk sizes (in ray blocks) for the three chunked s streams
    Q14 = SCHED[T - 3] // NQ  # 4
    QT = R_HALF // NQ  # 2

    PE_ALL = T  # 17
    PE_SCALAR = T + 1  # 18

    import contextlib

    with contextlib.ExitStack() as ctx:
        ec = ctx.enter_context
        w_sb = ec(nc.sbuf_tensor([P, NB * FREE], F32))
        m_sb = ec(nc.sbuf_tensor([P, NB * FREE], F32))
        s_sb = ec(nc.sbuf_tensor([P, NB * FREE], F32))
        # slot k holds wm in [k*2F, k*2F+F) and sw in [k*2F+F, k*2F+2F),
        # both contiguous; the matmul rhs is a [P, 2, N] strided view
        ws_sb = ec(nc.sbuf_tensor([P, NB * 2 * FREE], BF16))
        wb_sb = ec(nc.sbuf_tensor([P, NB * FREE], BF16))
        ai_sb = ec(nc.sbuf_tensor([P, 2 * N + 1], F32))
        acc_sb = ec(nc.sbuf_tensor([P, 1], F32))
        outs_sb = ec(nc.sbuf_tensor([1, 1], F32))
        tr_sb = ec(nc.sbuf_tensor([P, 2 * N], F32))
        g12_ps = ec(nc.psum_tensor([P, 2 * N], F32))  # [W^T WM | W^T SW]
        sc_ps = ec(nc.psum_tensor([1, 1], F32))
        w_sl = [ec(nc.semaphore(f"dma_w{i}")) for i in range(NB)]
        m_sl = [ec(nc.semaphore(f"dma_m{i}")) for i in range(NB)]
        s_sl = [ec(nc.semaphore(f"dma_s{i}")) for i in range(NB)]
        ai_sem = ec(nc.semaphore("dma_ai"))
        dve_sem = ec(nc.semaphore("dve_sem"))
        pe_sem = ec(nc.semaphore("pe_sem"))
        block = ec(nc.Block(no_gpsimd_drain=True))

        def rnd(i):
            # full-transfer threshold for tile i's w/m/s on its slot sem
            return 16 * (i // NB + 1)

        def sl(i, blk0=0, nblk=None):
            base = (i % NB) * FREE + blk0 * N
            n_el = (SCHED[i] if nblk is None else nblk) * N
            return slice(base, base + n_el)

        def ws_half(i, half, blk0=0, nblk=None):
            # contiguous [P, nblk*N] view of the slot's wm or sw half
            base = (i % NB) * 2 * FREE + half * FREE + blk0 * N
            n_el = (SCHED[i] if nblk is None else nblk) * N
            return ws_sb[:, base : base + n_el]

        def rhs_blk(i, r):
            # [P, 2, N] strided view: (wm_r ; sw_r) of block r in slot i%NB
            base2 = (i % NB) * 2 * FREE
            v = ws_sb[:, base2 : base2 + 2 * FREE].rearrange(
                "p (two f) -> p two f", two=2
            )
            return v[:, :, r * N : (r + 1) * N]

        @block.gpsimd
        def _(g: bass.BassEngine):
            # constants ride the SWDGE ring: zero cost to the HWDGE stream
            g.dma_start(out=ai_sb[:], in_=ai_h[:, :]).then_inc(ai_sem, 16)

        @block.sync
        def _(sync: bass.BassEngine):
            for i in range(T - 2):
                k = i % NB
                if i >= NB:
                    # io ring slot (i-NB) fully consumed by DVE
                    sync.wait_ge(dve_sem, dve_after_tile(i - NB))
                sync.dma_start(out=w_sb[:, sl(i)], in_=dram_view(w_h, i)).then_inc(
                    w_sl[k], 16
                )
                sync.dma_start(out=m_sb[:, sl(i)], in_=dram_view(m_h, i)).then_inc(
                    m_sl[k], 16
                )
                if i == T - 3:
                    # tile 14's s is chunked so the PE chase starts early
                    s_view = dram_view(s_h, i)
                    for q in range(NQ):
                        sync.dma_start(
                            out=s_sb[:, sl(i, q * Q14, Q14)],
                            in_=s_view[:, q * Q14 * N : (q + 1) * Q14 * N],
                        ).then_inc(s_sl[k], 16)
                else:
                    sync.dma_start(out=s_sb[:, sl(i)], in_=dram_view(s_h, i)).then_inc(
                        s_sl[k], 16
                    )
            # tail tiles 15/16: w and m ride the queue ahead of the s
            # chunks, so the only data landing at the stream end is s.
            for i in (T - 2, T - 1):
                k = i % NB
                sync.wait_ge(dve_sem, dve_after_tile(i - NB))
                sync.dma_start(out=w_sb[:, sl(i)], in_=dram_view(w_h, i)).then_inc(
                    w_sl[k], 16
                )
                sync.dma_start(out=m_sb[:, sl(i)], in_=dram_view(m_h, i)).then_inc(
                    m_sl[k], 16
                )
            for i in (T - 2, T - 1):
                k = i % NB
                s_view = dram_view(s_h, i)
                for q in range(NQ):
                    sync.dma_start(
                        out=s_sb[:, sl(i, q * QT, QT)],
                        in_=s_view[:, q * QT * N : (q + 1) * QT * N],
                    ).then_inc(s_sl[k], 16)
            sync.wait_ge(dve_sem, DVE_FINAL)
            sync.dma_start(out=out_h[:, :], in_=outs_sb[:]).then_inc(pe_sem, 16)
            # the out-DMA must fully land before the NEFF ends: an in-flight
            # DMA across the NEFF boundary corrupts runtime state.
            sync.wait_ge(pe_sem, PE_SCALAR + 16)

        @block.vector
        def _(vector: bass.BassEngine):
            for i in range(T - 3):
                k = i % NB
                if i >= NB:
                    # bf16 ring slot (i-NB) fully consumed by PE
                    vector.wait_ge(pe_sem, i - NB + 1)
                vector.wait_ge(w_sl[k], rnd(i))
                vector.tensor_copy(out=wb_sb[:, sl(i)], in_=w_sb[:, sl(i)]).then_inc(
                    dve_sem, 1
                )
                vector.wait_ge(m_sl[k], rnd(i))
                vector.tensor_mul(
                    ws_half(i, 0), w_sb[:, sl(i)], m_sb[:, sl(i)]
                ).then_inc(dve_sem, 1)
                vector.wait_ge(s_sl[k], rnd(i))
                vector.tensor_mul(
                    ws_half(i, 1), s_sb[:, sl(i)], w_sb[:, sl(i)]
                ).then_inc(dve_sem, 1)
            # tile 14: s is chunked
            i = T - 3
            k = i % NB
            vector.wait_ge(pe_sem, i - NB + 1)
            vector.wait_ge(w_sl[k], rnd(i))
            vector.tensor_copy(out=wb_sb[:, sl(i)], in_=w_sb[:, sl(i)]).then_inc(
                dve_sem, 1
            )
            vector.wait_ge(m_sl[k], rnd(i))
            vector.tensor_mul(ws_half(i, 0), w_sb[:, sl(i)], m_sb[:, sl(i)]).then_inc(
                dve_sem, 1
            )
            for q in range(NQ):
                vector.wait_ge(s_sl[k], 16 * (i // NB) + 16 * (q + 1))
                vector.tensor_mul(
                    ws_half(i, 1, q * Q14, Q14),
                    s_sb[:, sl(i, q * Q14, Q14)],
                    w_sb[:, sl(i, q * Q14, Q14)],
                ).then_inc(dve_sem, 1)
            # tail tiles: cast + wm as soon as their (early) loads land
            for i in (T - 2, T - 1):
                k = i % NB
                vector.wait_ge(pe_sem, i - NB + 1)
                vector.wait_ge(w_sl[k], rnd(i))
                vector.tensor_copy(out=wb_sb[:, sl(i)], in_=w_sb[:, sl(i)]).then_inc(
                    dve_sem, 1
                )
                vector.wait_ge(m_sl[k], rnd(i))
                vector.tensor_mul(
                    ws_half(i, 0), w_sb[:, sl(i)], m_sb[:, sl(i)]
                ).then_inc(dve_sem, 1)
            # chase the s chunks
            for i in (T - 2, T - 1):
                k = i % NB
                for q in range(NQ):
                    vector.wait_ge(s_sl[k], 16 * (i // NB) + 16 * (q + 1))
                    vector.tensor_mul(
                        ws_half(i, 1, q * QT, QT),
                        s_sb[:, sl(i, q * QT, QT)],
                        w_sb[:, sl(i, q * QT, QT)],
                    ).then_inc(dve_sem, 1)
            # finale: (G * [2A | I/3]) multiply-reduce -> [128,1]
            # (tensor_tensor_reduce would fuse these but fails codegen:
            # "ISA wrong length")
            vector.wait_ge(pe_sem, PE_ALL)
            vector.wait_ge(ai_sem, 16)
            vector.tensor_mul(tr_sb[:], g12_ps[:], ai_sb[:, 0 : 2 * N])
            vector.tensor_reduce(
                acc_sb[:],
                tr_sb[:],
                axis=mybir.AxisListType.X,
                op=mybir.AluOpType.add,
            ).then_inc(dve_sem, 1)
            # collapse to one scalar via the PE, then stage it for the DMA
            vector.wait_ge(pe_sem, PE_SCALAR)
            vector.tensor_copy(out=outs_sb[:], in_=sc_ps[:]).then_inc(dve_sem, 1)

        @block.tensor
        def _(tensor: bass.BassEngine):
            for i in range(T - 3):
                base = (i % NB) * FREE
                # one matmul per ray block needs cast + wm + sw (3 incs)
                tensor.wait_ge(dve_sem, 3 * i + 3)
                last_mm = None
                for r in range(SCHED[i]):
                    wblk = slice(base + r * N, base + (r + 1) * N)
                    last_mm = nc.tensor.matmul(
                        out=g12_ps[:],
                        lhsT=wb_sb[:, wblk],
                        rhs=rhs_blk(i, r),
                        start=(i == 0 and r == 0),
                        stop=False,
                    )
                last_mm.then_inc(pe_sem, 1)
            # tile 14 + tail tiles: chase the DVE chunks
            chase = [
                (T - 3, Q14, DVE_T14 + 2),
                (T - 2, QT, DVE_SW15),
                (T - 1, QT, DVE_SW15 + NQ),
            ]
            for i, qblk, sw_base in chase:
                base = (i % NB) * FREE
                last_mm = None
                prev_thr = -1
                for r in range(SCHED[i]):
                    thr = sw_base + r // qblk + 1
                    if thr != prev_thr:
                        tensor.wait_ge(dve_sem, thr)
                        prev_thr = thr
                    wblk = slice(base + r * N, base + (r + 1) * N)
                    last_mm = nc.tensor.matmul(
                        out=g12_ps[:],
                        lhsT=wb_sb[:, wblk],
                        rhs=rhs_blk(i, r),
                        start=False,
                        stop=(i == T - 1 and r == SCHED[i] - 1),
                    )
                last_mm.then_inc(pe_sem, 1)
            # ones-weighted column sum: [1,1] scalar in PSUM
            tensor.wait_ge(dve_sem, DVE_RED)
            nc.tensor.matmul(
                out=sc_ps[:],
                lhsT=acc_sb[:],
                rhs=ai_sb[:, 2 * N : 2 * N + 1],
                start=True,
                stop=True,
            ).then_inc(pe_sem, 1)

    return nc


def _aimat() -> np.ndarray:
    # transpose of (SL - SU): the kernel accumulates W^T WM = G1^T, and
    # <A, G1> = <A^T, G1^T>. The 2x (bi) and 1/3 (uni) loss weights are
    # folded in; the trailing column of ones drives the scalar-collapse
    # matmul.
    a = np.triu(np.ones((N, N), np.float32), 1) - np.tril(
        np.ones((N, N), np.float32), -1
    )
    return np.ascontiguousarray(
        np.concatenate(
            [
                2.0 * a,
                np.eye(N, dtype=np.float32) / 3.0,
                np.ones((N, 1), dtype=np.float32),
            ],
            axis=1,
        )
    )


def kernel(weights: np.ndarray, distances: np.ndarray, intervals: np.ndarray):
    if "nc" not in _cached:
        _cached["nc"] = _build_nc()
    nc = _cached["nc"]

    w8 = np.ascontiguousarray(weights, np.float32).reshape(NCORES, B_PER, N)
    m8 = np.ascontiguousarray(distances, np.float32).reshape(NCORES, B_PER, N)
    s8 = np.ascontiguousarray(intervals, np.float32).reshape(NCORES, B_PER, N)
    ai = _aimat()

    in_maps = [
        {
            "weights": w8[i],
            "distances": m8[i],
            "intervals": s8[i],
            "aimat": ai,
        }
        for i in range(NCORES)
    ]
    res = run_bass_kernel_spmd(nc, in_maps, list(range(NCORES))).results

    total = 0.0
    for i in range(NCORES):
        total += float(res[i]["partials"].astype(np.float64)[0, 0])

    loss = LOSS_WEIGHT * total / B
    return np.asarray(loss, dtype=np.float32)


# revision 5
# speedup vs baseline: 1.1986x; 1.0027x over previous
"""Distortion-loss (eff_distloss) Bass kernel for Trainium2, 8 NeuronCores.

Inputs (full): weights/distances/intervals, each [262144, 128] f32.
Output: scalar f32 loss.

Math: per ray (w, m, s in R^128):
  uni = sum_j s_j w_j^2
  bi  = sum_{j>k} w_j w_k (m_j - m_k) = wm^T (SL - SU) w,  wm = w*m,
        SL/SU strictly lower/upper triangular ones.
  loss = 0.01 * mean_rays(uni/3 + 2*bi)

Total bi over a batch of rays = <A^T, W^T WM>_F with A = SL - SU (constant)
and W^T WM a Gram matrix accumulated over rays; uni = sum diag(W^T SW),
sw = s*w. On the PE, each 128-ray block is ONE ldweights (stationary w) +
ONE 256-wide matmul streaming [wm ; sw] (a [P,2,N] strided rhs over the
slot's contiguous wm/sw halves -- strided DVE writes into an interleaved
layout run at ~96 G elem/s vs ~216 contiguous) into a single [128, 256]
PSUM accumulator holding both Gram matrices. The 2x (bi) and 1/3 (uni)
loss weights are folded into the constant matrix, so the finale is one
multiply+reduce into a [128,1] column, a ones-column matmul collapsing it
to a single scalar, and a 4-byte store (a [128,x] store fans into 128
tiny descriptors whose 16 per-engine completion incs straggle ~2us).

Sharding: pure data-parallel over the ray axis, B=262144 -> 32768 rays on
each of the 8 cores; the host sums the 8 scalars.

Raw-bass implementation (no Tile): engine programs (sync=HWDGE DMA,
vector=elementwise bf16 products, tensor=Gram matmuls, gpsimd=constant
load via the separate SWDGE ring). DMA completion uses one counting
semaphore per (tensor, ring slot) with full-transfer thresholds, so the
DVE starts a tile's cast/wm as soon as w/m land instead of waiting for
the whole tile. Tail: the last three tiles' s streams are chunked (the
w/m of the final two ride the FIFO queue ahead of all s chunks) so the
DVE/PE chase starts ~8us before the stream ends and only a ~1-block
tail remains after the last byte."""

import numpy as np

import concourse.bass as bass
import concourse.mybir as mybir
from concourse.bass_utils import run_bass_kernel_spmd

B, N = 262144, 128
NCORES = 8
B_PER = B // NCORES  # 32768 rays per core
P = 128  # SBUF partitions = rays per matmul block
RMAX = 16  # rays per partition in a full tile
# 15 full tiles + 2 half tiles = 15*16 + 2*8 = 256 ray-blocks per core
SCHED = [16] * 15 + [8, 8]
assert sum(SCHED) * P == B_PER
T = len(SCHED)
FREE = RMAX * N  # ring slot size (f32 elements per partition)
NB = 4  # ring depth
NQ = 4  # s-stream chunks for each of the last three tiles

F32 = mybir.dt.float32
BF16 = mybir.dt.bfloat16

LOSS_WEIGHT = 0.01

_cached = {}


def _build_nc() -> bass.Bass:
    nc = bass.Bass(trn_type="TRN2", monotonic_sem_count=0)

    w_h = nc.declare_dram_parameter("weights", [B_PER, N], F32, isOutput=False)
    m_h = nc.declare_dram_parameter("distances", [B_PER, N], F32, isOutput=False)
    s_h = nc.declare_dram_parameter("intervals", [B_PER, N], F32, isOutput=False)
    ai_h = nc.declare_dram_parameter("aimat", [P, 2 * N + 1], F32, isOutput=False)
    out_h = nc.declare_dram_parameter("partials", [1, 1], F32, isOutput=True)

    # per-tile DRAM views: tile i covers rays [off, off + P*R_i)
    offs = [0]
    for r in SCHED:
        offs.append(offs[-1] + P * r)

    def dram_view(h, i):
        r = SCHED[i]
        return h[offs[i] : offs[i + 1], :].rearrange("(p r) n -> p (r n)", p=P, r=r)

    # DVE inc ledger:
    #   tiles 0..13 : cast+wm+sw, 3 each                  -> 1..42
    #   tile 14     : cast, wm, sw chunks 0..3            -> 43..48
    #   cast15, wm15, cast16, wm16                        -> 49..52
    #   sw15 chunks 0..3, sw16 chunks 0..3                -> 53..60
    #   finale reduce                                     -> 61
    #   psum-scalar copy                                  -> 62
    def dve_after_tile(i):
        assert i <= T - 4
        return 3 * (i + 1)

    DVE_T14 = 3 * (T - 3)  # 42: count before tile 14's ops
    DVE_TAIL = DVE_T14 + 2 + NQ  # 48: count before cast15
    DVE_SW15 = DVE_TAIL + 4  # 52: count before sw15 chunk 0
    DVE_RED = DVE_SW15 + 2 * NQ + 1  # 61: the finale reduce's inc
    DVE_FINAL = DVE_RED + 1  # 62

    R_HALF = SCHED[-1]  # 8 blocks in each tail tile
    # chunk sizes (in ray blocks) for the three chunked s streams
    Q14 = SCHED[T - 3] // NQ  # 4
    QT = R_HALF // NQ  # 2

    PE_ALL = T  # 17
    PE_SCALAR = T + 1  # 18

    import contextlib

    with contextlib.ExitStack() as ctx:
        ec = ctx.enter_context
        w_sb = ec(nc.sbuf_tensor([P, NB * FREE], F32))
        m_sb = ec(nc.sbuf_tensor([P, NB * FREE], F32))
        s_sb = ec(nc.sbuf_tensor([P, NB * FREE], F32))
        # slot k holds wm in [k*2F, k*2F+F) and sw in [k*2F+F, k*2F+2F),
        # both contiguous; the matmul rhs is a [P, 2, N] strided view
        ws_sb = ec(nc.sbuf_tensor([P, NB * 2 * FREE], BF16))
        wb_sb = ec(nc.sbuf_tensor([P, NB * FREE], BF16))
        ai_sb = ec(nc.sbuf_tensor([P, 2 * N + 1], F32))
        acc_sb = ec(nc.sbuf_tensor([P, 1], F32))
        outs_sb = ec(nc.sbuf_tensor([1, 1], F32))
        tr_sb = ec(nc.sbuf_tensor([P, 2 * N], F32))
        g12_ps = ec(nc.psum_tensor([P, 2 * N], F32))  # [W^T WM | W^T SW]
        sc_ps = ec(nc.psum_tensor([1, 1], F32))
        w_sl = [ec(nc.semaphore(f"dma_w{i}")) for i in range(NB)]
        m_sl = [ec(nc.semaphore(f"dma_m{i}")) for i in range(NB)]
        s_sl = [ec(nc.semaphore(f"dma_s{i}")) for i in range(NB)]
        ai_sem = ec(nc.semaphore("dma_ai"))
        dve_sem = ec(nc.semaphore("dve_sem"))
        pe_sem = ec(nc.semaphore("pe_sem"))
        block = ec(nc.Block(no_gpsimd_drain=True))

        def rnd(i):
            # full-transfer threshold for tile i's w/m/s on its slot sem
            return 16 * (i // NB + 1)

        def sl(i, blk0=0, nblk=None):
            base = (i % NB) * FREE + blk0 * N
            n_el = (SCHED[i] if nblk is None else nblk) * N
            return slice(base, base + n_el)

        def ws_half(i, half, blk0=0, nblk=None):
            # contiguous [P, nblk*N] view of the slot's wm or sw half
            base = (i % NB) * 2 * FREE + half * FREE + blk0 * N
            n_el = (SCHED[i] if nblk is None else nblk) * N
            return ws_sb[:, base : base + n_el]

        def rhs_blk(i, r):
            # [P, 2, N] strided view: (wm_r ; sw_r) of block r in slot i%NB
            base2 = (i % NB) * 2 * FREE
            v = ws_sb[:, base2 : base2 + 2 * FREE].rearrange(
                "p (two f) -> p two f", two=2
            )
            return v[:, :, r * N : (r + 1) * N]

        @block.scalar
        def _(act: bass.BassEngine):
            # constants ride the second HWDGE queue (qAct): off the sync
            # queue's FIFO, lands at stream start. NOTE: must NOT use the
            # gpsimd SWDGE path -- a single SWDGE DMA in the NEFF halves
            # DVE throughput (2-port perf mode conflicts with the SWDGE
            # descriptor rings in SBUF).
            act.dma_start(out=ai_sb[:], in_=ai_h[:, :]).then_inc(ai_sem, 16)

        @block.sync
        def _(sync: bass.BassEngine):
            for i in range(T - 2):
                k = i % NB
                if i >= NB:
                    # io ring slot (i-NB) fully consumed by DVE
                    sync.wait_ge(dve_sem, dve_after_tile(i - NB))
                sync.dma_start(out=w_sb[:, sl(i)], in_=dram_view(w_h, i)).then_inc(
                    w_sl[k], 16
                )
                sync.dma_start(out=m_sb[:, sl(i)], in_=dram_view(m_h, i)).then_inc(
                    m_sl[k], 16
                )
                if i == T - 3:
                    # tile 14's s is chunked so the PE chase starts early
                    s_view = dram_view(s_h, i)
                    for q in range(NQ):
                        sync.dma_start(
                            out=s_sb[:, sl(i, q * Q14, Q14)],
                            in_=s_view[:, q * Q14 * N : (q + 1) * Q14 * N],
                        ).then_inc(s_sl[k], 16)
                else:
                    sync.dma_start(out=s_sb[:, sl(i)], in_=dram_view(s_h, i)).then_inc(
                        s_sl[k], 16
                    )
            # tail tiles 15/16: w and m ride the queue ahead of the s
            # chunks, so the only data landing at the stream end is s.
            for i in (T - 2, T - 1):
                k = i % NB
                sync.wait_ge(dve_sem, dve_after_tile(i - NB))
                sync.dma_start(out=w_sb[:, sl(i)], in_=dram_view(w_h, i)).then_inc(
                    w_sl[k], 16
                )
                sync.dma_start(out=m_sb[:, sl(i)], in_=dram_view(m_h, i)).then_inc(
                    m_sl[k], 16
                )
            for i in (T - 2, T - 1):
                k = i % NB
                s_view = dram_view(s_h, i)
                for q in range(NQ):
                    sync.dma_start(
                        out=s_sb[:, sl(i, q * QT, QT)],
                        in_=s_view[:, q * QT * N : (q + 1) * QT * N],
                    ).then_inc(s_sl[k], 16)
            sync.wait_ge(dve_sem, DVE_FINAL)
            sync.dma_start(out=out_h[:, :], in_=outs_sb[:]).then_inc(pe_sem, 16)
            # the out-DMA must fully land before the NEFF ends: an in-flight
            # DMA across the NEFF boundary corrupts runtime state.
            sync.wait_ge(pe_sem, PE_SCALAR + 16)

        @block.vector
        def _(vector: bass.BassEngine):
            for i in range(T - 3):
                k = i % NB
                if i >= NB:
                    # bf16 ring slot (i-NB) fully consumed by PE
                    vector.wait_ge(pe_sem, i - NB + 1)
                vector.wait_ge(w_sl[k], rnd(i))
                vector.tensor_copy(out=wb_sb[:, sl(i)], in_=w_sb[:, sl(i)]).then_inc(
                    dve_sem, 1
                )
                vector.wait_ge(m_sl[k], rnd(i))
                vector.tensor_mul(
                    ws_half(i, 0), w_sb[:, sl(i)], m_sb[:, sl(i)]
                ).then_inc(dve_sem, 1)
                vector.wait_ge(s_sl[k], rnd(i))
                vector.tensor_mul(
                    ws_half(i, 1), s_sb[:, sl(i)], w_sb[:, sl(i)]
                ).then_inc(dve_sem, 1)
            # tile 14: s is chunked
            i = T - 3
            k = i % NB
            vector.wait_ge(pe_sem, i - NB + 1)
            vector.wait_ge(w_sl[k], rnd(i))
            vector.tensor_copy(out=wb_sb[:, sl(i)], in_=w_sb[:, sl(i)]).then_inc(
                dve_sem, 1
            )
            vector.wait_ge(m_sl[k], rnd(i))
            vector.tensor_mul(ws_half(i, 0), w_sb[:, sl(i)], m_sb[:, sl(i)]).then_inc(
                dve_sem, 1
            )
            for q in range(NQ):
                vector.wait_ge(s_sl[k], 16 * (i // NB) + 16 * (q + 1))
                vector.tensor_mul(
                    ws_half(i, 1, q * Q14, Q14),
                    s_sb[:, sl(i, q * Q14, Q14)],
                    w_sb[:, sl(i, q * Q14, Q14)],
                ).then_inc(dve_sem, 1)
            # tail tiles: cast + wm as soon as their (early) loads land
            for i in (T - 2, T - 1):
                k = i % NB
                vector.wait_ge(pe_sem, i - NB + 1)
                vector.wait_ge(w_sl[k], rnd(i))
                vector.tensor_copy(out=wb_sb[:, sl(i)], in_=w_sb[:, sl(i)]).then_inc(
                    dve_sem, 1
                )
                vector.wait_ge(m_sl[k], rnd(i))
                vector.tensor_mul(
                    ws_half(i, 0), w_sb[:, sl(i)], m_sb[:, sl(i)]
                ).then_inc(dve_sem, 1)
            # chase the s chunks
            for i in (T - 2, T - 1):
                k = i % NB
                for q in range(NQ):
                    vector.wait_ge(s_sl[k], 16 * (i // NB) + 16 * (q + 1))
                    vector.tensor_mul(
                        ws_half(i, 1, q * QT, QT),
                        s_sb[:, sl(i, q * QT, QT)],
                        w_sb[:, sl(i, q * QT, QT)],
                    ).then_inc(dve_sem, 1)
            # finale: (G * [2A | I/3]) multiply-reduce -> [128,1]
            # (tensor_tensor_reduce would fuse these but fails codegen:
            # "ISA wrong length")
            vector.wait_ge(pe_sem, PE_ALL)
            vector.wait_ge(ai_sem, 16)
            vector.tensor_mul(tr_sb[:], g12_ps[:], ai_sb[:, 0 : 2 * N])
            vector.tensor_reduce(
                acc_sb[:],
                tr_sb[:],
                axis=mybir.AxisListType.X,
                op=mybir.AluOpType.add,
            ).then_inc(dve_sem, 1)
            # collapse to one scalar via the PE, then stage it for the DMA
            vector.wait_ge(pe_sem, PE_SCALAR)
            vector.tensor_copy(out=outs_sb[:], in_=sc_ps[:]).then_inc(dve_sem, 1)

        @block.tensor
        def _(tensor: bass.BassEngine):
            for i in range(T - 3):
                base = (i % NB) * FREE
                # one matmul per ray block needs cast + wm + sw (3 incs)
                tensor.wait_ge(dve_sem, 3 * i + 3)
                last_mm = None
                for r in range(SCHED[i]):
                    wblk = slice(base + r * N, base + (r + 1) * N)
                    last_mm = nc.tensor.matmul(
                        out=g12_ps[:],
                        lhsT=wb_sb[:, wblk],
                        rhs=rhs_blk(i, r),
                        start=(i == 0 and r == 0),
                        stop=False,
                    )
                last_mm.then_inc(pe_sem, 1)
            # tile 14 + tail tiles: chase the DVE chunks
            chase = [
                (T - 3, Q14, DVE_T14 + 2),
                (T - 2, QT, DVE_SW15),
                (T - 1, QT, DVE_SW15 + NQ),
            ]
            for i, qblk, sw_base in chase:
                base = (i % NB) * FREE
                last_mm = None
                prev_thr = -1
                for r in range(SCHED[i]):
                    thr = sw_base + r // qblk + 1
                    if thr != prev_thr:
                        tensor.wait_ge(dve_sem, thr)
                        prev_thr = thr
                    wblk = slice(base + r * N, base + (r + 1) * N)
                    last_mm = nc.tensor.matmul(
                        out=g12_ps[:],
                        lhsT=wb_sb[:, wblk],
                        rhs=rhs_blk(i, r),
                        start=False,
                        stop=(i == T - 1 and r == SCHED[i] - 1),
                    )
                last_mm.then_inc(pe_sem, 1)
            # ones-weighted column sum: [1,1] scalar in PSUM
            tensor.wait_ge(dve_sem, DVE_RED)
            nc.tensor.matmul(
                out=sc_ps[:],
                lhsT=acc_sb[:],
                rhs=ai_sb[:, 2 * N : 2 * N + 1],
                start=True,
                stop=True,
            ).then_inc(pe_sem, 1)

    return nc


def _aimat() -> np.ndarray:
    # transpose of (SL - SU): the kernel accumulates W^T WM = G1^T, and
    # <A, G1> = <A^T, G1^T>. The 2x (bi) and 1/3 (uni) loss weights are
    # folded in; the trailing column of ones drives the scalar-collapse
    # matmul.
    a = np.triu(np.ones((N, N), np.float32), 1) - np.tril(
        np.ones((N, N), np.float32), -1
    )
    return np.ascontiguousarray(
        np.concatenate(
            [
                2.0 * a,
                np.eye(N, dtype=np.float32) / 3.0,
                np.ones((N, 1), dtype=np.float32),
            ],
            axis=1,
        )
    )


def kernel(weights: np.ndarray, distances: np.ndarray, intervals: np.ndarray):
    if "nc" not in _cached:
        _cached["nc"] = _build_nc()
    nc = _cached["nc"]

    w8 = np.ascontiguousarray(weights, np.float32).reshape(NCORES, B_PER, N)
    m8 = np.ascontiguousarray(distances, np.float32).reshape(NCORES, B_PER, N)
    s8 = np.ascontiguousarray(intervals, np.float32).reshape(NCORES, B_PER, N)
    ai = _aimat()

    in_maps = [
        {
            "weights": w8[i],
            "distances": m8[i],
            "intervals": s8[i],
            "aimat": ai,
        }
        for i in range(NCORES)
    ]
    res = run_bass_kernel_spmd(nc, in_maps, list(range(NCORES))).results

    total = 0.0
    for i in range(NCORES):
        total += float(res[i]["partials"].astype(np.float64)[0, 0])

    loss = LOSS_WEIGHT * total / B
    return np.asarray(loss, dtype=np.float32)


# revision 6
# speedup vs baseline: 1.2417x; 1.0360x over previous
"""Distortion-loss (eff_distloss) Bass kernel for Trainium2, 8 NeuronCores.

Inputs (full): weights/distances/intervals, each [262144, 128] f32.
Output: scalar f32 loss.

Math: per ray (w, m, s in R^128):
  uni = sum_j s_j w_j^2
  bi  = sum_{j>k} w_j w_k (m_j - m_k) = wm^T (SL - SU) w,  wm = w*m,
        SL/SU strictly lower/upper triangular ones.
  loss = 0.01 * mean_rays(uni/3 + 2*bi)

Total bi over a batch of rays = <A^T, W^T WM>_F with A = SL - SU (constant)
and W^T WM a Gram matrix accumulated over rays; uni = sum diag(W^T SW),
sw = s*w. On the PE, each 128-ray block is ONE ldweights (stationary w) +
ONE 256-wide matmul streaming [wm ; sw] (a [P,2,N] strided rhs over the
slot's contiguous wm/sw halves) into a single [128, 256] PSUM accumulator
holding both Gram matrices. The 2x (bi) and 1/3 (uni) loss weights are
folded into the constant matrix, so the finale is one multiply+reduce
into a [128,1] column, a ones-column matmul collapsing it to a single
scalar, and a 4-byte store (a [128,x] store fans into 128 tiny
descriptors whose 16 per-engine completion incs straggle ~2us).

Sharding: pure data-parallel over the ray axis, B=262144 -> 32768 rays on
each of the 8 cores; the host sums the 8 scalars.

Engine split (raw bass, no Tile):
  gpsimd : w loads as SWDGE cast-DMAs (f32 HBM -> bf16 SBUF). Same HBM
           read bytes, but no DVE cast op -- DVE per-tile work drops from
           ~0.89x of the stream rate to ~0.71x, which is what lets the
           tail drain instead of bunching.
  sync   : m/s loads (HWDGE), the final 4-byte store.
  scalar : the constant-matrix load on the second HWDGE queue.
  vector : wm/sw products (bf16 w times f32 m/s -> bf16), the finale.
  tensor : Gram matmuls, the ones-column scalar collapse.
DMA completion uses one counting semaphore per (tensor, ring slot) with
full-transfer thresholds so the DVE starts wm the moment w+m land. The
last 4 tiles are 4 blocks each in dedicated (non-ring) buffers issued
with no ring guards, so the queue never starves at the tail and the
post-stream chain is a couple of small ops + finale."""

import numpy as np

import concourse.bass as bass
import concourse.mybir as mybir
from concourse.bass_utils import run_bass_kernel_spmd

B, N = 262144, 128
NCORES = 8
B_PER = B // NCORES  # 32768 rays per core
P = 128  # SBUF partitions = rays per matmul block
RMAX = 16  # rays per partition in a full tile
# 15 full ring tiles + 4 dedicated tail tiles of 4 blocks
SCHED = [16] * 15 + [4, 4, 4, 4]
assert sum(SCHED) * P == B_PER
T = len(SCHED)
NRING = 15  # tiles that live in the ring
FREE = RMAX * N  # ring slot size (f32 elements per partition)
NB = 4  # ring depth
TBLK = sum(SCHED[NRING:])  # 16 tail blocks
TFREE = TBLK * N

F32 = mybir.dt.float32
BF16 = mybir.dt.bfloat16

LOSS_WEIGHT = 0.01

_cached = {}


def _build_nc() -> bass.Bass:
    nc = bass.Bass(trn_type="TRN2", monotonic_sem_count=0)

    w_h = nc.declare_dram_parameter("weights", [B_PER, N], F32, isOutput=False)
    m_h = nc.declare_dram_parameter("distances", [B_PER, N], F32, isOutput=False)
    s_h = nc.declare_dram_parameter("intervals", [B_PER, N], F32, isOutput=False)
    ai_h = nc.declare_dram_parameter("aimat", [P, 2 * N + 1], F32, isOutput=False)
    out_h = nc.declare_dram_parameter("partials", [1, 1], F32, isOutput=True)

    # per-tile DRAM views: tile i covers rays [off, off + P*R_i)
    offs = [0]
    for r in SCHED:
        offs.append(offs[-1] + P * r)

    def dram_view(h, i):
        r = SCHED[i]
        return h[offs[i] : offs[i + 1], :].rearrange("(p r) n -> p (r n)", p=P, r=r)

    # tail tile j starts at block toff[j] of the tail buffers
    toff = [0]
    for r in SCHED[NRING:]:
        toff.append(toff[-1] + r)

    # DVE inc ledger: wm+sw per tile -> 2 each, then finale reduce, then
    # the psum-scalar copy.
    def dve_after_tile(i):
        return 2 * (i + 1)

    DVE_RED = 2 * T + 1  # 39
    DVE_FINAL = DVE_RED + 1  # 40

    PE_ALL = T  # 19
    PE_SCALAR = T + 1  # 20

    import contextlib

    with contextlib.ExitStack() as ctx:
        ec = ctx.enter_context
        wb_sb = ec(nc.sbuf_tensor([P, NB * FREE], BF16))
        m_sb = ec(nc.sbuf_tensor([P, NB * FREE], F32))
        s_sb = ec(nc.sbuf_tensor([P, NB * FREE], F32))
        # slot k holds wm in [k*2F, k*2F+F) and sw in [k*2F+F, k*2F+2F),
        # both contiguous; the matmul rhs is a [P, 2, N] strided view
        ws_sb = ec(nc.sbuf_tensor([P, NB * 2 * FREE], BF16))
        # dedicated tail buffers (no ring reuse, no guards)
        wb_tl = ec(nc.sbuf_tensor([P, TFREE], BF16))
        m_tl = ec(nc.sbuf_tensor([P, TFREE], F32))
        s_tl = ec(nc.sbuf_tensor([P, TFREE], F32))
        ws_tl = ec(nc.sbuf_tensor([P, 2 * TFREE], BF16))
        ai_sb = ec(nc.sbuf_tensor([P, 2 * N + 1], F32))
        acc_sb = ec(nc.sbuf_tensor([P, 1], F32))
        outs_sb = ec(nc.sbuf_tensor([1, 1], F32))
        tr_sb = ec(nc.sbuf_tensor([P, 2 * N], F32))
        g12_ps = ec(nc.psum_tensor([P, 2 * N], F32))  # [W^T WM | W^T SW]
        sc_ps = ec(nc.psum_tensor([1, 1], F32))
        w_sl = [ec(nc.semaphore(f"dma_w{i}")) for i in range(NB)]
        m_sl = [ec(nc.semaphore(f"dma_m{i}")) for i in range(NB)]
        s_sl = [ec(nc.semaphore(f"dma_s{i}")) for i in range(NB)]
        w_tsem = ec(nc.semaphore("dma_wt"))
        m_tsem = ec(nc.semaphore("dma_mt"))
        s_tsem = ec(nc.semaphore("dma_st"))
        ai_sem = ec(nc.semaphore("dma_ai"))
        dve_sem = ec(nc.semaphore("dve_sem"))
        pe_sem = ec(nc.semaphore("pe_sem"))
        block = ec(nc.Block(no_gpsimd_drain=True))

        def rnd(i):
            # full-transfer threshold for ring tile i on its slot sem
            return 16 * (i // NB + 1)

        def sl(i):
            base = (i % NB) * FREE
            return slice(base, base + SCHED[i] * N)

        def tsl(i):
            j = i - NRING
            return slice(toff[j] * N, toff[j + 1] * N)

        def wm_dst(i):
            if i < NRING:
                base = (i % NB) * 2 * FREE
                return ws_sb[:, base : base + SCHED[i] * N]
            j = i - NRING
            return ws_tl[:, toff[j] * N : toff[j + 1] * N]

        def sw_dst(i):
            if i < NRING:
                base = (i % NB) * 2 * FREE + FREE
                return ws_sb[:, base : base + SCHED[i] * N]
            j = i - NRING
            return ws_tl[:, TFREE + toff[j] * N : TFREE + toff[j + 1] * N]

        def rhs_blk(i, r):
            # [P, 2, N] strided view: (wm_r ; sw_r) of block r
            if i < NRING:
                base2 = (i % NB) * 2 * FREE
                v = ws_sb[:, base2 : base2 + 2 * FREE].rearrange(
                    "p (two f) -> p two f", two=2
                )
            else:
                v = ws_tl[:].rearrange("p (two f) -> p two f", two=2)
                r = toff[i - NRING] + r
            return v[:, :, r * N : (r + 1) * N]

        def lhs_blk(i, r):
            if i < NRING:
                base = (i % NB) * FREE
                return wb_sb[:, base + r * N : base + (r + 1) * N]
            r = toff[i - NRING] + r
            return wb_tl[:, r * N : (r + 1) * N]

        @block.scalar
        def _(act: bass.BassEngine):
            # constants ride the second HWDGE queue: off the sync queue's
            # FIFO, lands at stream start
            act.dma_start(out=ai_sb[:], in_=ai_h[:, :]).then_inc(ai_sem, 16)

        @block.gpsimd
        def _(g: bass.BassEngine):
            # w rides the SWDGE queue as cast-DMAs (f32 -> bf16): same HBM
            # reads, no DVE cast op, and w needs only half the SBUF
            for i in range(NRING):
                if i >= NB:
                    # wb ring slot (i-NB) fully consumed by the PE
                    g.wait_ge(pe_sem, i - NB + 1)
                g.dma_start(out=wb_sb[:, sl(i)], in_=dram_view(w_h, i)).then_inc(
                    w_sl[i % NB], 16
                )
            for i in range(NRING, T):
                g.dma_start(out=wb_tl[:, tsl(i)], in_=dram_view(w_h, i)).then_inc(
                    w_tsem, 16
                )

        @block.sync
        def _(sync: bass.BassEngine):
            for i in range(NRING):
                k = i % NB
                if i >= NB:
                    # m/s ring slot (i-NB) fully consumed by DVE
                    sync.wait_ge(dve_sem, dve_after_tile(i - NB))
                sync.dma_start(out=m_sb[:, sl(i)], in_=dram_view(m_h, i)).then_inc(
                    m_sl[k], 16
                )
                sync.dma_start(out=s_sb[:, sl(i)], in_=dram_view(s_h, i)).then_inc(
                    s_sl[k], 16
                )
            for i in range(NRING, T):
                sync.dma_start(out=m_tl[:, tsl(i)], in_=dram_view(m_h, i)).then_inc(
                    m_tsem, 16
                )
                sync.dma_start(out=s_tl[:, tsl(i)], in_=dram_view(s_h, i)).then_inc(
                    s_tsem, 16
                )
            sync.wait_ge(dve_sem, DVE_FINAL)
            sync.dma_start(out=out_h[:, :], in_=outs_sb[:]).then_inc(pe_sem, 16)
            # the out-DMA must fully land before the NEFF ends: an in-flight
            # DMA across the NEFF boundary corrupts runtime state.
            sync.wait_ge(pe_sem, PE_SCALAR + 16)

        @block.vector
        def _(vector: bass.BassEngine):
            for i in range(NRING):
                k = i % NB
                if i >= NB:
                    # bf16 ws ring slot (i-NB) fully consumed by PE
                    vector.wait_ge(pe_sem, i - NB + 1)
                vector.wait_ge(w_sl[k], rnd(i))
                vector.wait_ge(m_sl[k], rnd(i))
                vector.tensor_mul(wm_dst(i), wb_sb[:, sl(i)], m_sb[:, sl(i)]).then_inc(
                    dve_sem, 1
                )
                vector.wait_ge(s_sl[k], rnd(i))
                vector.tensor_mul(sw_dst(i), s_sb[:, sl(i)], wb_sb[:, sl(i)]).then_inc(
                    dve_sem, 1
                )
            for i in range(NRING, T):
                j = i - NRING
                vector.wait_ge(w_tsem, 16 * (j + 1))
                vector.wait_ge(m_tsem, 16 * (j + 1))
                vector.tensor_mul(wm_dst(i), wb_tl[:, tsl(i)], m_tl[:, tsl(i)]).then_inc(
                    dve_sem, 1
                )
                vector.wait_ge(s_tsem, 16 * (j + 1))
                vector.tensor_mul(sw_dst(i), s_tl[:, tsl(i)], wb_tl[:, tsl(i)]).then_inc(
                    dve_sem, 1
                )
            # finale: (G * [2A | I/3]) multiply-reduce -> [128,1]
            # (tensor_tensor_reduce would fuse these but fails codegen)
            vector.wait_ge(pe_sem, PE_ALL)
            vector.wait_ge(ai_sem, 16)
            vector.tensor_mul(tr_sb[:], g12_ps[:], ai_sb[:, 0 : 2 * N])
            vector.tensor_reduce(
                acc_sb[:],
                tr_sb[:],
                axis=mybir.AxisListType.X,
                op=mybir.AluOpType.add,
            ).then_inc(dve_sem, 1)
            # collapse to one scalar via the PE, then stage it for the DMA
            vector.wait_ge(pe_sem, PE_SCALAR)
            vector.tensor_copy(out=outs_sb[:], in_=sc_ps[:]).then_inc(dve_sem, 1)

        @block.tensor
        def _(tensor: bass.BassEngine):
            for i in range(T):
                # one matmul per ray block; the tile's wm+sw must be done
                tensor.wait_ge(dve_sem, 2 * i + 2)
                last_mm = None
                for r in range(SCHED[i]):
                    last_mm = nc.tensor.matmul(
                        out=g12_ps[:],
                        lhsT=lhs_blk(i, r),
                        rhs=rhs_blk(i, r),
                        start=(i == 0 and r == 0),
                        stop=(i == T - 1 and r == SCHED[i] - 1),
                    )
                last_mm.then_inc(pe_sem, 1)
            # ones-weighted column sum: [1,1] scalar in PSUM
            tensor.wait_ge(dve_sem, DVE_RED)
            nc.tensor.matmul(
                out=sc_ps[:],
                lhsT=acc_sb[:],
                rhs=ai_sb[:, 2 * N : 2 * N + 1],
                start=True,
                stop=True,
            ).then_inc(pe_sem, 1)

    return nc


def _aimat() -> np.ndarray:
    # transpose of (SL - SU): the kernel accumulates W^T WM = G1^T, and
    # <A, G1> = <A^T, G1^T>. The 2x (bi) and 1/3 (uni) loss weights are
    # folded in; the trailing column of ones drives the scalar-collapse
    # matmul.
    a = np.triu(np.ones((N, N), np.float32), 1) - np.tril(
        np.ones((N, N), np.float32), -1
    )
    return np.ascontiguousarray(
        np.concatenate(
            [
                2.0 * a,
                np.eye(N, dtype=np.float32) / 3.0,
                np.ones((N, 1), dtype=np.float32),
            ],
            axis=1,
        )
    )


def kernel(weights: np.ndarray, distances: np.ndarray, intervals: np.ndarray):
    if "nc" not in _cached:
        _cached["nc"] = _build_nc()
    nc = _cached["nc"]

    w8 = np.ascontiguousarray(weights, np.float32).reshape(NCORES, B_PER, N)
    m8 = np.ascontiguousarray(distances, np.float32).reshape(NCORES, B_PER, N)
    s8 = np.ascontiguousarray(intervals, np.float32).reshape(NCORES, B_PER, N)
    ai = _aimat()

    in_maps = [
        {
            "weights": w8[i],
            "distances": m8[i],
            "intervals": s8[i],
            "aimat": ai,
        }
        for i in range(NCORES)
    ]
    res = run_bass_kernel_spmd(nc, in_maps, list(range(NCORES))).results

    total = 0.0
    for i in range(NCORES):
        total += float(res[i]["partials"].astype(np.float64)[0, 0])

    loss = LOSS_WEIGHT * total / B
    return np.asarray(loss, dtype=np.float32)


# revision 7
# speedup vs baseline: 1.2517x; 1.0080x over previous
"""Distortion-loss (eff_distloss) Bass kernel for Trainium2, 8 NeuronCores.

Inputs (full): weights/distances/intervals, each [262144, 128] f32.
Output: scalar f32 loss.

Math: per ray (w, m, s in R^128):
  uni = sum_j s_j w_j^2
  bi  = sum_{j>k} w_j w_k (m_j - m_k) = wm^T (SL - SU) w,  wm = w*m,
        SL/SU strictly lower/upper triangular ones.
  loss = 0.01 * mean_rays(uni/3 + 2*bi)

Total bi over a batch of rays = <A^T, W^T WM>_F with A = SL - SU (constant)
and W^T WM a Gram matrix accumulated over rays; uni = sum diag(W^T SW),
sw = s*w. On the PE, each 128-ray block is ONE ldweights (stationary w) +
ONE 256-wide matmul streaming [wm ; sw] (a [P,2,N] strided rhs over the
slot's contiguous wm/sw halves) into a single [128, 256] PSUM accumulator
holding both Gram matrices. The 2x (bi) and 1/3 (uni) loss weights are
folded into the constant matrix, so the finale is one multiply+reduce
into a [128,1] column, a ones-column matmul collapsing it to a single
scalar, and a 4-byte store (a [128,x] store fans into 128 tiny
descriptors whose 16 per-engine completion incs straggle ~2us).

Sharding: pure data-parallel over the ray axis, B=262144 -> 32768 rays on
each of the 8 cores; the host sums the 8 scalars.

Engine split (raw bass, no Tile):
  gpsimd : w loads as SWDGE cast-DMAs (f32 HBM -> bf16 SBUF). Same HBM
           read bytes, but no DVE cast op -- DVE per-tile work drops from
           ~0.89x of the stream rate to ~0.71x, which is what lets the
           tail drain instead of bunching.
  sync   : m/s loads (HWDGE), the final 4-byte store.
  scalar : the constant-matrix load on the second HWDGE queue.
  vector : wm/sw products (bf16 w times f32 m/s -> bf16), the finale.
  tensor : Gram matmuls, the ones-column scalar collapse.
DMA completion uses one counting semaphore per (tensor, ring slot) with
full-transfer thresholds so the DVE starts wm the moment w+m land. The
last 4 tiles are 4 blocks each in dedicated (non-ring) buffers issued
with no ring guards, so the queue never starves at the tail and the
post-stream chain is a couple of small ops + finale."""

import numpy as np

import concourse.bass as bass
import concourse.mybir as mybir
from concourse.bass_utils import run_bass_kernel_spmd

B, N = 262144, 128
NCORES = 8
B_PER = B // NCORES  # 32768 rays per core
P = 128  # SBUF partitions = rays per matmul block
RMAX = 16  # rays per partition in a full tile
# 15 full ring tiles + dedicated tail tiles (progressively smaller)
SCHED = [16] * 15 + [4, 4, 4, 2, 2]
assert sum(SCHED) * P == B_PER
T = len(SCHED)
NRING = 15  # tiles that live in the ring
FREE = RMAX * N  # ring slot size (f32 elements per partition)
NB = 4  # ring depth
TBLK = sum(SCHED[NRING:])  # 16 tail blocks
TFREE = TBLK * N

F32 = mybir.dt.float32
BF16 = mybir.dt.bfloat16

LOSS_WEIGHT = 0.01

_cached = {}


def _build_nc() -> bass.Bass:
    nc = bass.Bass(trn_type="TRN2", monotonic_sem_count=0)

    w_h = nc.declare_dram_parameter("weights", [B_PER, N], F32, isOutput=False)
    m_h = nc.declare_dram_parameter("distances", [B_PER, N], F32, isOutput=False)
    s_h = nc.declare_dram_parameter("intervals", [B_PER, N], F32, isOutput=False)
    ai_h = nc.declare_dram_parameter("aimat", [P, 2 * N + 1], F32, isOutput=False)
    out_h = nc.declare_dram_parameter("partials", [1, 1], F32, isOutput=True)

    # per-tile DRAM views: tile i covers rays [off, off + P*R_i)
    offs = [0]
    for r in SCHED:
        offs.append(offs[-1] + P * r)

    def dram_view(h, i):
        r = SCHED[i]
        return h[offs[i] : offs[i + 1], :].rearrange("(p r) n -> p (r n)", p=P, r=r)

    # tail tile j starts at block toff[j] of the tail buffers
    toff = [0]
    for r in SCHED[NRING:]:
        toff.append(toff[-1] + r)

    # DVE inc ledger: wm+sw per tile -> 2 each, then finale reduce, then
    # the psum-scalar copy.
    def dve_after_tile(i):
        return 2 * (i + 1)

    DVE_RED = 2 * T + 1  # 39
    DVE_FINAL = DVE_RED + 1  # 40

    PE_ALL = T  # 19
    PE_SCALAR = T + 1  # 20

    import contextlib

    with contextlib.ExitStack() as ctx:
        ec = ctx.enter_context
        wb_sb = ec(nc.sbuf_tensor([P, NB * FREE], BF16))
        m_sb = ec(nc.sbuf_tensor([P, NB * FREE], F32))
        s_sb = ec(nc.sbuf_tensor([P, NB * FREE], F32))
        # slot k holds wm in [k*2F, k*2F+F) and sw in [k*2F+F, k*2F+2F),
        # both contiguous; the matmul rhs is a [P, 2, N] strided view
        ws_sb = ec(nc.sbuf_tensor([P, NB * 2 * FREE], BF16))
        # dedicated tail buffers (no ring reuse, no guards)
        wb_tl = ec(nc.sbuf_tensor([P, TFREE], BF16))
        m_tl = ec(nc.sbuf_tensor([P, TFREE], F32))
        s_tl = ec(nc.sbuf_tensor([P, TFREE], F32))
        ws_tl = ec(nc.sbuf_tensor([P, 2 * TFREE], BF16))
        ai_sb = ec(nc.sbuf_tensor([P, 2 * N + 1], F32))
        acc_sb = ec(nc.sbuf_tensor([P, 1], F32))
        outs_sb = ec(nc.sbuf_tensor([1, 1], F32))
        tr_sb = ec(nc.sbuf_tensor([P, 2 * N], F32))
        g12_ps = ec(nc.psum_tensor([P, 2 * N], F32))  # [W^T WM | W^T SW]
        sc_ps = ec(nc.psum_tensor([1, 1], F32))
        w_sl = [ec(nc.semaphore(f"dma_w{i}")) for i in range(NB)]
        m_sl = [ec(nc.semaphore(f"dma_m{i}")) for i in range(NB)]
        s_sl = [ec(nc.semaphore(f"dma_s{i}")) for i in range(NB)]
        w_tsem = ec(nc.semaphore("dma_wt"))
        m_tsem = ec(nc.semaphore("dma_mt"))
        s_tsem = ec(nc.semaphore("dma_st"))
        ai_sem = ec(nc.semaphore("dma_ai"))
        dve_sem = ec(nc.semaphore("dve_sem"))
        pe_sem = ec(nc.semaphore("pe_sem"))
        block = ec(nc.Block(no_gpsimd_drain=True))

        def rnd(i):
            # full-transfer threshold for ring tile i on its slot sem
            return 16 * (i // NB + 1)

        def sl(i):
            base = (i % NB) * FREE
            return slice(base, base + SCHED[i] * N)

        def tsl(i):
            j = i - NRING
            return slice(toff[j] * N, toff[j + 1] * N)

        def wm_dst(i):
            if i < NRING:
                base = (i % NB) * 2 * FREE
                return ws_sb[:, base : base + SCHED[i] * N]
            j = i - NRING
            return ws_tl[:, toff[j] * N : toff[j + 1] * N]

        def sw_dst(i):
            if i < NRING:
                base = (i % NB) * 2 * FREE + FREE
                return ws_sb[:, base : base + SCHED[i] * N]
            j = i - NRING
            return ws_tl[:, TFREE + toff[j] * N : TFREE + toff[j + 1] * N]

        def rhs_blk(i, r):
            # [P, 2, N] strided view: (wm_r ; sw_r) of block r
            if i < NRING:
                base2 = (i % NB) * 2 * FREE
                v = ws_sb[:, base2 : base2 + 2 * FREE].rearrange(
                    "p (two f) -> p two f", two=2
                )
            else:
                v = ws_tl[:].rearrange("p (two f) -> p two f", two=2)
                r = toff[i - NRING] + r
            return v[:, :, r * N : (r + 1) * N]

        def lhs_blk(i, r):
            if i < NRING:
                base = (i % NB) * FREE
                return wb_sb[:, base + r * N : base + (r + 1) * N]
            r = toff[i - NRING] + r
            return wb_tl[:, r * N : (r + 1) * N]

        @block.scalar
        def _(act: bass.BassEngine):
            # constants ride the second HWDGE queue: off the sync queue's
            # FIFO, lands at stream start
            act.dma_start(out=ai_sb[:], in_=ai_h[:, :]).then_inc(ai_sem, 16)

        @block.gpsimd
        def _(g: bass.BassEngine):
            # w rides the SWDGE queue as cast-DMAs (f32 -> bf16): same HBM
            # reads, no DVE cast op, and w needs only half the SBUF
            for i in range(NRING):
                if i >= NB:
                    # wb ring slot (i-NB) fully consumed by the PE
                    g.wait_ge(pe_sem, i - NB + 1)
                g.dma_start(out=wb_sb[:, sl(i)], in_=dram_view(w_h, i)).then_inc(
                    w_sl[i % NB], 16
                )
            for i in range(NRING, T):
                g.dma_start(out=wb_tl[:, tsl(i)], in_=dram_view(w_h, i)).then_inc(
                    w_tsem, 16
                )

        @block.sync
        def _(sync: bass.BassEngine):
            for i in range(NRING):
                k = i % NB
                if i >= NB:
                    # m/s ring slot (i-NB) fully consumed by DVE
                    sync.wait_ge(dve_sem, dve_after_tile(i - NB))
                sync.dma_start(out=m_sb[:, sl(i)], in_=dram_view(m_h, i)).then_inc(
                    m_sl[k], 16
                )
                sync.dma_start(out=s_sb[:, sl(i)], in_=dram_view(s_h, i)).then_inc(
                    s_sl[k], 16
                )
            for i in range(NRING, T):
                sync.dma_start(out=m_tl[:, tsl(i)], in_=dram_view(m_h, i)).then_inc(
                    m_tsem, 16
                )
                sync.dma_start(out=s_tl[:, tsl(i)], in_=dram_view(s_h, i)).then_inc(
                    s_tsem, 16
                )
            sync.wait_ge(dve_sem, DVE_FINAL)
            sync.dma_start(out=out_h[:, :], in_=outs_sb[:]).then_inc(pe_sem, 16)
            # the out-DMA must fully land before the NEFF ends: an in-flight
            # DMA across the NEFF boundary corrupts runtime state.
            sync.wait_ge(pe_sem, PE_SCALAR + 16)

        @block.vector
        def _(vector: bass.BassEngine):
            for i in range(NRING):
                k = i % NB
                if i >= NB:
                    # bf16 ws ring slot (i-NB) fully consumed by PE
                    vector.wait_ge(pe_sem, i - NB + 1)
                vector.wait_ge(w_sl[k], rnd(i))
                vector.wait_ge(m_sl[k], rnd(i))
                vector.tensor_mul(wm_dst(i), wb_sb[:, sl(i)], m_sb[:, sl(i)]).then_inc(
                    dve_sem, 1
                )
                vector.wait_ge(s_sl[k], rnd(i))
                vector.tensor_mul(sw_dst(i), s_sb[:, sl(i)], wb_sb[:, sl(i)]).then_inc(
                    dve_sem, 1
                )
            for i in range(NRING, T):
                j = i - NRING
                vector.wait_ge(w_tsem, 16 * (j + 1))
                vector.wait_ge(m_tsem, 16 * (j + 1))
                vector.tensor_mul(wm_dst(i), wb_tl[:, tsl(i)], m_tl[:, tsl(i)]).then_inc(
                    dve_sem, 1
                )
                vector.wait_ge(s_tsem, 16 * (j + 1))
                vector.tensor_mul(sw_dst(i), s_tl[:, tsl(i)], wb_tl[:, tsl(i)]).then_inc(
                    dve_sem, 1
                )
            # finale: (G * [2A | I/3]) multiply-reduce -> [128,1]
            # (tensor_tensor_reduce would fuse these but fails codegen)
            vector.wait_ge(pe_sem, PE_ALL)
            vector.wait_ge(ai_sem, 16)
            vector.tensor_mul(tr_sb[:], g12_ps[:], ai_sb[:, 0 : 2 * N])
            vector.tensor_reduce(
                acc_sb[:],
                tr_sb[:],
                axis=mybir.AxisListType.X,
                op=mybir.AluOpType.add,
            ).then_inc(dve_sem, 1)
            # collapse to one scalar via the PE, then stage it for the DMA
            vector.wait_ge(pe_sem, PE_SCALAR)
            vector.tensor_copy(out=outs_sb[:], in_=sc_ps[:]).then_inc(dve_sem, 1)

        @block.tensor
        def _(tensor: bass.BassEngine):
            for i in range(T):
                # one matmul per ray block; the tile's wm+sw must be done
                tensor.wait_ge(dve_sem, 2 * i + 2)
                last_mm = None
                for r in range(SCHED[i]):
                    last_mm = nc.tensor.matmul(
                        out=g12_ps[:],
                        lhsT=lhs_blk(i, r),
                        rhs=rhs_blk(i, r),
                        start=(i == 0 and r == 0),
                        stop=(i == T - 1 and r == SCHED[i] - 1),
                    )
                last_mm.then_inc(pe_sem, 1)
            # ones-weighted column sum: [1,1] scalar in PSUM
            tensor.wait_ge(dve_sem, DVE_RED)
            nc.tensor.matmul(
                out=sc_ps[:],
                lhsT=acc_sb[:],
                rhs=ai_sb[:, 2 * N : 2 * N + 1],
                start=True,
                stop=True,
            ).then_inc(pe_sem, 1)

    return nc


def _aimat() -> np.ndarray:
    # transpose of (SL - SU): the kernel accumulates W^T WM = G1^T, and
    # <A, G1> = <A^T, G1^T>. The 2x (bi) and 1/3 (uni) loss weights are
    # folded in; the trailing column of ones drives the scalar-collapse
    # matmul.
    a = np.triu(np.ones((N, N), np.float32), 1) - np.tril(
        np.ones((N, N), np.float32), -1
    )
    return np.ascontiguousarray(
        np.concatenate(
            [
                2.0 * a,
                np.eye(N, dtype=np.float32) / 3.0,
                np.ones((N, 1), dtype=np.float32),
            ],
            axis=1,
        )
    )


def kernel(weights: np.ndarray, distances: np.ndarray, intervals: np.ndarray):
    if "nc" not in _cached:
        _cached["nc"] = _build_nc()
    nc = _cached["nc"]

    w8 = np.ascontiguousarray(weights, np.float32).reshape(NCORES, B_PER, N)
    m8 = np.ascontiguousarray(distances, np.float32).reshape(NCORES, B_PER, N)
    s8 = np.ascontiguousarray(intervals, np.float32).reshape(NCORES, B_PER, N)
    ai = _aimat()

    in_maps = [
        {
            "weights": w8[i],
            "distances": m8[i],
            "intervals": s8[i],
            "aimat": ai,
        }
        for i in range(NCORES)
    ]
    res = run_bass_kernel_spmd(nc, in_maps, list(range(NCORES))).results

    total = 0.0
    for i in range(NCORES):
        total += float(res[i]["partials"].astype(np.float64)[0, 0])

    loss = LOSS_WEIGHT * total / B
    return np.asarray(loss, dtype=np.float32)
